# revision 3
# baseline (speedup 1.0000x reference)
"""HardTripletMiningLoss on 8 TRN2 NeuronCores (Bass, raw-block SPMD).

Math: with emb = concat(anchor, positive, negative) [N,D], labels = ind[:,0],
pd(a,b) = ||e_a - e_b||^2, the loss is the mean over triplets (i,j,k) of
td = pd(i,j) - pd(j,k) + A restricted to
  same(i,j) & ~same(j,k) & td > 0 & i != 0.
Only (i,j) pairs with same labels (and i>=1) contribute — ~N^2/L of N^2 pairs.
Each such pair p=(i,j) needs, over k: sum/count of relu(td), where
  td(p,k) = 2*g(j,k) - n_k + (n_i - 2*u_p + A),  u_p = <e_i, e_j>,
and same-label k are masked by adding -BIG inside the matmul accumulation.

Per core: pairs become rows of [128, N] tiles; PE computes
  V'[p,k] = g(j_p,k) - n_k/2 - (BIG/2)*same(j_p,k)
via two accumulating matmuls (emb^T gathered by j, then a [1+L, .] aux matmul
whose row 0 carries -n_k/2 and rows 1..L carry the one-hot label mask), then
ACT does relu(2*V' + bias_p) with a fused row-sum and DVE does the fused
count (V' > -bias_p/2). Host sums the 8 cores' partials and forms the mean.
"""

import numpy as np
from contextlib import ExitStack

import concourse.bass as bass
import concourse.mybir as mybir
from concourse.bass_utils import run_bass_kernel_spmd

F32 = mybir.dt.float32
AF = mybir.ActivationFunctionType
OP = mybir.AluOpType

N_CORES = 8
A_MARGIN = 0.2
BIG_HALF = 1.0e6  # BIG = 2e6 dominates any |td| (~1e3) by >>1e3x
PAD_NEG = -1.0e9  # bias for padding pair rows -> relu 0, count 0
MAX_TILES = 4     # per-core pair tiles per launch (PSUM bank budget)

_programs: dict = {}  # (T, N, L) -> bass.Bass
LAST_RES = None  # most recent BassKernelResults (for test harness tracing)


def _build_program(T: int, N: int, L: int) -> "bass.Bass":
    """One SPMD program: every core runs this with its own pair shard."""
    P = T * 128
    nc = bass.Bass()

    d_embt = nc.declare_dram_parameter("embt", [128, N], F32, isOutput=False)
    d_embjt = nc.declare_dram_parameter("embjt", [128, P], F32, isOutput=False)
    d_embit = nc.declare_dram_parameter("embit", [128, P], F32, isOutput=False)
    d_auxl = nc.declare_dram_parameter("auxl", [1 + L, P], F32, isOutput=False)
    d_auxrh = nc.declare_dram_parameter("auxrh", [L, N], F32, isOutput=False)
    d_padcol = nc.declare_dram_parameter("padcol", [128, T], F32, isOutput=False)
    d_out_s = nc.declare_dram_parameter("out_s", [128, T], F32, isOutput=True)
    d_out_c = nc.declare_dram_parameter("out_c", [128, T], F32, isOutput=True)

    with ExitStack() as ctx:
        sb = lambda name, shape: ctx.enter_context(nc.sbuf_tensor(name, shape, F32))
        ps = lambda name, shape: ctx.enter_context(nc.psum_tensor(name, shape, F32))

        embt_sb = sb("embt_sb", [128, N])
        embjt_sb = sb("embjt_sb", [128, P])
        embit_sb = sb("embit_sb", [128, P])
        auxl_sb = sb("auxl_sb", [1 + L, P])
        # row 0: -n_k/2 (device), rows 1..L: host mask
        auxr_sb = sb("auxr_sb", [1 + L, N])
        padcol_sb = sb("padcol_sb", [128, T])
        sq_t = sb("sq_t", [128, N])
        tmp_u = sb("tmp_u", [128, P])
        sqi = sb("sqi", [128, P])
        bias_row = sb("bias_row", [1, P])
        thresh_col = sb("thresh_col", [128, T])
        rs_col = sb("rs_col", [128, T])
        rc_col = sb("rc_col", [128, T])
        scratch_a = sb("scratch_a", [128, N])
        zeros_nt = sb("zeros_nt", [128, N])
        scratch_d = sb("scratch_d", [128, N])
        ones128 = sb("ones128", [128, 1])
        neg2_128 = sb("neg2_128", [128, 1])
        ones11 = sb("ones11", [1, 1])

        psum_n = ps("psum_n", [1, N])
        psum_u = ps("psum_u", [1, P])
        psum_bias = ps("psum_bias", [128, T])
        psumV = [ps(f"psumV{t}", [128, N]) for t in range(T)]

        with (
            nc.semaphore("dma_in") as dma_in,
            nc.semaphore("dma_out") as dma_out,
            nc.semaphore("v_pre") as v_pre,
            nc.semaphore("a_pre") as a_pre,
            nc.semaphore("pe_pre") as pe_pre,
            nc.semaphore("mm") as mm,
            nc.semaphore("dve_s") as dve_s,
            nc.Block() as block,
        ):

            @block.sync
            def _(sync):
                sync.dma_start(embt_sb[:], d_embt[:]).then_inc(dma_in, 16)
                sync.dma_start(embjt_sb[:], d_embjt[:]).then_inc(dma_in, 16)
                sync.dma_start(embit_sb[:], d_embit[:]).then_inc(dma_in, 16)
                sync.dma_start(auxl_sb[:], d_auxl[:]).then_inc(dma_in, 16)
                sync.dma_start(auxr_sb[1:1 + L, :], d_auxrh[:]).then_inc(dma_in, 16)
                sync.dma_start(padcol_sb[:], d_padcol[:]).then_inc(dma_in, 16)
                sync.wait_ge(dve_s, 2 * T)
                sync.dma_start(d_out_s[:], rs_col[:]).then_inc(dma_out, 16)
                sync.dma_start(d_out_c[:], rc_col[:]).then_inc(dma_out, 16)
                sync.wait_ge(dma_out, 32)

            @block.vector
            def _(vector):
                nc.vector.memset(ones128[:], 1.0).then_inc(v_pre, 1)   # 1
                nc.vector.memset(ones11[:], 1.0).then_inc(v_pre, 1)    # 2
                nc.vector.memset(neg2_128[:], -2.0).then_inc(v_pre, 1)  # 3
                nc.vector.memset(zeros_nt[:], 0.0).then_inc(v_pre, 1)  # 4
                vector.wait_ge(dma_in, 96)
                nc.vector.tensor_tensor(
                    sq_t[:], embt_sb[:], embt_sb[:], OP.mult
                ).then_inc(v_pre, 1)                                   # 5
                nc.vector.tensor_tensor(
                    tmp_u[:], embit_sb[:], embjt_sb[:], OP.mult
                ).then_inc(v_pre, 1)                                   # 6
                nc.vector.tensor_tensor(
                    sqi[:], embit_sb[:], embit_sb[:], OP.mult
                ).then_inc(v_pre, 1)                                   # 7
                vector.wait_ge(pe_pre, 2)
                # bias_row = -2*u + n_i accumulated in psum_u by PE
                nc.vector.tensor_copy(bias_row[:], psum_u[:]).then_inc(v_pre, 1)  # 8
                vector.wait_ge(pe_pre, 2 + T)
                # thresh = -(bias + pad)/2; host sends padcol pre-halved/negated
                nc.vector.scalar_tensor_tensor(
                    thresh_col[:], psum_bias[:], -0.5, padcol_sb[:], OP.mult, OP.add
                ).then_inc(v_pre, 1)                                   # 9
                vector.wait_ge(v_pre, 9)
                for t in range(T):
                    vector.wait_ge(mm, t + 1)
                    if t > 0:
                        vector.wait_ge(dve_s, 2 * t - 1)
                    # sum of relu((2V'+bias)/2) -> host multiplies by 2
                    nc.vector.scalar_tensor_tensor(
                        scratch_a[:], psumV[t][:], thresh_col[:, t:t + 1],
                        zeros_nt[:], OP.subtract, OP.max,
                        accum_out=rs_col[:, t:t + 1],
                    ).then_inc(dve_s, 1)
                    if t > 0:
                        vector.wait_ge(dve_s, 2 * t)
                    nc.vector.tensor_scalar(
                        scratch_d[:], psumV[t][:], thresh_col[:, t:t + 1], None,
                        OP.is_gt, OP.add, accum_out=rc_col[:, t:t + 1],
                    ).then_inc(dve_s, 1)

            @block.scalar
            def _(scalar):
                scalar.wait_ge(pe_pre, 1)
                nc.scalar.mul(auxr_sb[0:1, :], psum_n[0:1, :], -0.5).then_inc(a_pre, 1)

            @block.tensor
            def _(tensor):
                tensor.wait_ge(v_pre, 5)
                nc.tensor.matmul(
                    psum_n[:], ones128[:], sq_t[:], start=True, stop=True
                ).then_inc(pe_pre, 1)
                tensor.wait_ge(v_pre, 7)
                nc.tensor.matmul(
                    psum_u[:], neg2_128[:], tmp_u[:], start=True, stop=False
                )
                nc.tensor.matmul(
                    psum_u[:], ones128[:], sqi[:], start=False, stop=True
                ).then_inc(pe_pre, 1)
                tensor.wait_ge(v_pre, 8)
                for t in range(T):
                    # transpose bias_row chunk t -> psum_bias[:, t]
                    nc.tensor.matmul(
                        psum_bias[:, t:t + 1], bias_row[0:1, bass.ts(t, 128)],
                        ones11[:], start=True, stop=True,
                    ).then_inc(pe_pre, 1)
                tensor.wait_ge(dma_in, 96)
                tensor.wait_ge(a_pre, 1)
                for t in range(T):
                    nc.tensor.matmul(
                        psumV[t][:], embjt_sb[:, bass.ts(t, 128)], embt_sb[:],
                        start=True, stop=False,
                    )
                    nc.tensor.matmul(
                        psumV[t][:], auxl_sb[:, bass.ts(t, 128)], auxr_sb[:],
                        start=False, stop=True,
                    ).then_inc(mm, 1)

    return nc


def _get_program(T: int, N: int, L: int) -> "bass.Bass":
    key = (T, N, L)
    if key not in _programs:
        _programs[key] = _build_program(T, N, L)
    return _programs[key]


def _run_batch(emb, labels, sq_labels_masks, ii, jj, T):
    """Run one SPMD launch over <=8*T*128 pairs; returns (sum, count) f64."""
    N, D = emb.shape
    L, embt, auxrh = sq_labels_masks
    P = T * 128
    per = (len(ii) + N_CORES - 1) // N_CORES

    in_maps = []
    for c in range(N_CORES):
        si = ii[c * per:(c + 1) * per]
        sj = jj[c * per:(c + 1) * per]
        m = len(si)
        embjt = np.zeros((D, P), np.float32)
        embit = np.zeros((D, P), np.float32)
        auxl = np.zeros((1 + L, P), np.float32)
        flat_pad = np.full(P, -0.5 * PAD_NEG, np.float32)
        if m:
            embjt[:, :m] = emb[sj].T
            embit[:, :m] = emb[si].T
            auxl[0, :m] = 1.0
            auxl[1 + labels[sj], np.arange(m)] = 1.0
            flat_pad[:m] = -0.5 * A_MARGIN
        padcol = np.ascontiguousarray(flat_pad.reshape(T, 128).T)
        in_maps.append({
            "embt": embt,
            "embjt": embjt,
            "embit": embit,
            "auxl": auxl,
            "auxrh": auxrh,
            "padcol": padcol,
        })

    nc = _get_program(T, N, L)
    res = run_bass_kernel_spmd(nc, in_maps, list(range(N_CORES)))
    global LAST_RES
    LAST_RES = res
    s = 0.0
    cnt = 0.0
    for c in range(N_CORES):
        s += 2.0 * float(res.results[c]["out_s"].astype(np.float64).sum())
        cnt += float(res.results[c]["out_c"].astype(np.float64).sum())
    return s, cnt


def kernel(anchor, positive, negative, ind):
    anchor = np.asarray(anchor, dtype=np.float32)
    positive = np.asarray(positive, dtype=np.float32)
    negative = np.asarray(negative, dtype=np.float32)
    labels = np.asarray(ind).reshape(-1).astype(np.int64)

    emb = np.ascontiguousarray(np.concatenate([anchor, positive, negative], axis=0))
    N, D = emb.shape
    assert D == 128, f"kernel assumes D=128, got {D}"
    assert N == labels.shape[0]

    L = int(labels.max()) + 1 if labels.size else 1
    assert L <= 127, f"label ids must fit one-hot partitions, got {L}"

    # same-label (i, j) pairs, excluding the i=0 plane (keep[0] = False)
    same = labels[:, None] == labels[None, :]
    ii, jj = np.nonzero(same)
    sel = ii >= 1
    ii, jj = ii[sel].astype(np.int64), jj[sel].astype(np.int64)

    if len(ii) == 0:
        return np.float32(0.0)

    embt = np.ascontiguousarray(emb.T)
    auxrh = np.zeros((L, N), np.float32)
    auxrh[labels, np.arange(N)] = -BIG_HALF
    shared = (L, embt, auxrh)

    batch_cap = N_CORES * MAX_TILES * 128
    s_tot, c_tot = 0.0, 0.0
    for b0 in range(0, len(ii), batch_cap):
        bi, bj = ii[b0:b0 + batch_cap], jj[b0:b0 + batch_cap]
        per = (len(bi) + N_CORES - 1) // N_CORES
        T = max(1, (per + 127) // 128)
        s, c = _run_batch(emb, labels, shared, bi, bj, T)
        s_tot += s
        c_tot += c

    if c_tot > 0:
        return np.float32(s_tot / max(c_tot, 1.0))
    return np.float32(0.0)



# revision 26
# speedup vs baseline: 1.9408x; 1.9408x over previous
"""HardTripletMiningLoss on 8 TRN2 NeuronCores (Bass, raw-block SPMD).

Math: with emb = concat(anchor, positive, negative) [N,D], labels = ind[:,0],
pd(a,b) = ||e_a - e_b||^2, the loss is the mean over triplets (i,j,k) of
td = pd(i,j) - pd(j,k) + A restricted to
  same(i,j) & ~same(j,k) & td > 0 & i != 0.
Only (i,j) pairs with same labels (and i>=1) contribute — ~N^2/L of N^2 pairs.
Each such pair p=(i,j) needs, over k: sum/count of relu(td), where
  td(p,k) = 2*g(j,k) - n_k + (n_i - 2*u_p + A),  u_p = <e_i, e_j>.

Device work per core (pairs sharded 8 ways, rows of [128, N] tiles):
  V[p,k] = g(j_p,k) - (n_k - mean_n)/2 - (BIG/2)*same(j_p,k)
via two accumulating bf16 matmuls per tile (emb^T gathered by j against
emb^T, then a one-hot label lhsT against an aux rhs whose columns carry
-(n_k - mean_n)/2 - BIG/2*onehot). The per-pair constant
  halfbias_p = (n_i - 2*u_p + A - mean_n)/2
is computed on HOST (O(pairs*D) numpy, same order as the gathers) so that
  relu(td) = 2*relu(V + halfbias_p)   and   td > 0  <=>  V > -halfbias_p.
ACT (scalar engine) does the relu row-sums, DVE (vector engine) the counts,
in parallel, one tile behind PE. Host sums the 8 cores' partials.
"""

import numpy as np
import ml_dtypes
from contextlib import ExitStack

import concourse.bass as bass
import concourse.mybir as mybir
from concourse.bass_utils import run_bass_kernel_spmd

F32 = mybir.dt.float32
BF16 = mybir.dt.bfloat16
AF = mybir.ActivationFunctionType
OP = mybir.AluOpType
NP_BF16 = ml_dtypes.bfloat16

N_CORES = 8
A_MARGIN = 0.2
BIG_HALF = 1.0e6  # BIG = 2e6 dominates any |td| (~1e3) by >>1e3x
PAD_NEG = -1.0e9  # halfbias for padding pair rows -> relu 0, count 0
MAX_TILES = 4     # per-core pair tiles per launch (PSUM bank budget)

_programs: dict = {}  # (T, N, L) -> bass.Bass
LAST_RES = None  # most recent BassKernelResults (for test harness tracing)


def _build_program(T: int, N: int, L: int) -> "bass.Bass":
    """One SPMD program: every core runs this with its own pair shard."""
    P = T * 128
    nc = bass.Bass()

    d_big = nc.declare_dram_parameter("big", [128, N + P], BF16, isOutput=False)
    d_aux = nc.declare_dram_parameter("aux", [L, P + N], BF16, isOutput=False)
    d_hcol = nc.declare_dram_parameter("hcol", [128, 2 * T], F32, isOutput=False)
    d_red = nc.declare_dram_parameter("red", [128, 2 * T], F32, isOutput=True)

    with ExitStack() as ctx:
        sb = lambda name, shape, dt: ctx.enter_context(nc.sbuf_tensor(name, shape, dt))
        ps = lambda name, shape: ctx.enter_context(nc.psum_tensor(name, shape, F32))

        # cols 0:N = emb^T (rhs of mm1), cols N:N+P = emb^T gathered by j (lhsT)
        big_sb = sb("big_sb", [128, N + P], BF16)
        # cols 0:P = one-hot(label_j) (lhsT of mm2), cols P:P+N = aux rhs
        aux_sb = sb("aux_sb", [L, P + N], BF16)
        hcol_sb = sb("hcol_sb", [128, 2 * T], F32)  # halfbias | -halfbias
        red_sb = sb("red_sb", [128, 2 * T], F32)    # relu sums | counts
        scr_a = sb("scr_a", [128, N], BF16)
        scr_d = sb("scr_d", [128, N], BF16)
        warm_sb = sb("warm_sb", [128, 1], BF16)
        psumV = [ps(f"psumV{t}", [128, N]) for t in range(T)]

        with (
            nc.semaphore("dma_in") as dma_in,
            nc.semaphore("hc") as hc,
            nc.semaphore("mm") as mm,
            nc.semaphore("act") as act,
            nc.semaphore("dve") as dve,
            nc.semaphore("dma_out") as dma_out,
            nc.Block() as block,
        ):

            @block.sync
            def _(sync):
                sync.dma_start(hcol_sb[:], d_hcol[:]).then_inc(hc, 16)
                sync.dma_start(big_sb[:], d_big[:]).then_inc(dma_in, 16)
                sync.dma_start(aux_sb[:], d_aux[:]).then_inc(dma_in, 16)
                sync.wait_ge(act, T)
                sync.wait_ge(dve, T)
                sync.dma_start(d_red[:], red_sb[:]).then_inc(dma_out, 16)
                sync.wait_ge(dma_out, 16)

            @block.tensor
            def _(tensor):
                tensor.wait_ge(dma_in, 32)
                for t in range(T):
                    nc.tensor.matmul(
                        psumV[t][:], big_sb[:, N + 128 * t:N + 128 * (t + 1)],
                        big_sb[:, 0:N], start=True, stop=False,
                    )
                    nc.tensor.matmul(
                        psumV[t][:], aux_sb[:, bass.ts(t, 128)],
                        aux_sb[:, P:P + N], start=False, stop=True,
                    ).then_inc(mm, 1)

            @block.scalar
            def _(scalar):
                # warm the Relu table while the big DMAs are in flight
                scalar.wait_ge(hc, 16)
                nc.scalar.activation(warm_sb[:], hcol_sb[:, 0:1], AF.Relu)
                for t in range(T):
                    scalar.wait_ge(mm, t + 1)
                    if t:
                        scalar.wait_ge(act, t)
                    nc.scalar.activation(
                        scr_a[:], psumV[t][:], AF.Relu,
                        bias=hcol_sb[:, t:t + 1],
                        accum_out=red_sb[:, t:t + 1],
                    ).then_inc(act, 1)

            @block.vector
            def _(vector):
                # act >= t+1 transitively implies hcol arrived and mm >= t+1
                for t in range(T):
                    # HW breaks if ACT and DVE read the same PSUM bank
                    # concurrently; trail ACT by one tile (banks differ).
                    vector.wait_ge(act, t + 1)
                    if t:
                        vector.wait_ge(dve, t)
                    nc.vector.tensor_scalar(
                        scr_d[:], psumV[t][:], hcol_sb[:, T + t:T + t + 1],
                        None, OP.is_gt, OP.add,
                        accum_out=red_sb[:, T + t:T + t + 1],
                    ).then_inc(dve, 1)

    return nc


def _get_program(T: int, N: int, L: int) -> "bass.Bass":
    key = (T, N, L)
    if key not in _programs:
        _programs[key] = _build_program(T, N, L)
    return _programs[key]


def _run_batch(shared, ii, jj, halfbias, T):
    """Run one SPMD launch over <=8*T*128 pairs; returns (sum, count) f64."""
    N, L, labels, emb_bf, embt_bf, auxr_bf = shared
    P = T * 128
    per = (len(ii) + N_CORES - 1) // N_CORES

    in_maps = []
    for c in range(N_CORES):
        sj = jj[c * per:(c + 1) * per]
        hb = halfbias[c * per:(c + 1) * per]
        m = len(sj)
        big = np.zeros((128, N + P), NP_BF16)
        big[:, :N] = embt_bf
        aux = np.zeros((L, P + N), NP_BF16)
        aux[:, P:] = auxr_bf
        hb_full = np.full(P, PAD_NEG, np.float32)
        if m:
            big[:, N:N + m] = emb_bf[sj].T
            aux[labels[sj], np.arange(m)] = 1.0
            hb_full[:m] = hb
        hcol = np.concatenate(
            [hb_full.reshape(T, 128).T, -hb_full.reshape(T, 128).T], axis=1
        )
        in_maps.append({
            "big": big,
            "aux": aux,
            "hcol": np.ascontiguousarray(hcol),
        })

    nc = _get_program(T, N, L)
    res = run_bass_kernel_spmd(nc, in_maps, list(range(N_CORES)))
    global LAST_RES
    LAST_RES = res
    s = 0.0
    cnt = 0.0
    for c in range(N_CORES):
        r = res.results[c]["red"].astype(np.float64)
        s += 2.0 * float(r[:, :T].sum())
        cnt += float(r[:, T:].sum())
    return s, cnt


def kernel(anchor, positive, negative, ind):
    anchor = np.asarray(anchor, dtype=np.float32)
    positive = np.asarray(positive, dtype=np.float32)
    negative = np.asarray(negative, dtype=np.float32)
    labels = np.asarray(ind).reshape(-1).astype(np.int64)

    emb = np.ascontiguousarray(np.concatenate([anchor, positive, negative], axis=0))
    N, D = emb.shape
    assert D == 128, f"kernel assumes D=128, got {D}"
    assert N == labels.shape[0]

    L = int(labels.max()) + 1 if labels.size else 1
    assert L <= 128, f"label ids must fit one-hot partitions, got {L}"

    # same-label (i, j) pairs, excluding the i=0 plane (keep[0] = False)
    same = labels[:, None] == labels[None, :]
    ii, jj = np.nonzero(same)
    sel = ii >= 1
    ii, jj = ii[sel].astype(np.int64), jj[sel].astype(np.int64)

    if len(ii) == 0:
        return np.float32(0.0)

    n = np.einsum("ij,ij->i", emb, emb, dtype=np.float64)
    mean_n = float(n.mean())
    u = np.einsum("ij,ij->i", emb[ii], emb[jj], dtype=np.float64)
    halfbias = ((n[ii] - 2.0 * u + A_MARGIN - mean_n) / 2.0).astype(np.float32)

    emb_bf = emb.astype(NP_BF16)
    embt_bf = np.ascontiguousarray(emb_bf.T)
    auxr = np.tile((-(n - mean_n) / 2.0).astype(np.float32), (L, 1))
    auxr[labels, np.arange(N)] -= BIG_HALF
    auxr_bf = auxr.astype(NP_BF16)
    shared = (N, L, labels, emb_bf, embt_bf, auxr_bf)

    batch_cap = N_CORES * MAX_TILES * 128
    s_tot, c_tot = 0.0, 0.0
    for b0 in range(0, len(ii), batch_cap):
        bi = ii[b0:b0 + batch_cap]
        bj = jj[b0:b0 + batch_cap]
        hb = halfbias[b0:b0 + batch_cap]
        per = (len(bi) + N_CORES - 1) // N_CORES
        T = max(1, (per + 127) // 128)
        s, c = _run_batch(shared, bi, bj, hb, T)
        s_tot += s
        c_tot += c

    if c_tot > 0:
        return np.float32(s_tot / max(c_tot, 1.0))
    return np.float32(0.0)


# revision 33
# speedup vs baseline: 1.9436x; 1.0015x over previous
"""HardTripletMiningLoss on 8 TRN2 NeuronCores (Bass, raw-block SPMD).

Math: with emb = concat(anchor, positive, negative) [N,D], labels = ind[:,0],
pd(a,b) = ||e_a - e_b||^2, the loss is the mean over triplets (i,j,k) of
td = pd(i,j) - pd(j,k) + A restricted to
  same(i,j) & ~same(j,k) & td > 0 & i != 0.
Only (i,j) pairs with same labels (and i>=1) contribute — ~N^2/L of N^2 pairs.
Each such pair p=(i,j) needs, over k: sum/count of relu(td), where
  td(p,k) = 2*g(j,k) - n_k + (n_i - 2*u_p + A),  u_p = <e_i, e_j>.

Device work per core (pairs sharded 8 ways, rows of [128, N] tiles):
  V[p,k] = g(j_p,k) - (n_k - mean_n)/2 - (BIG/2)*same(j_p,k)
via two accumulating bf16 matmuls per tile (emb^T gathered by j against
emb^T, then a one-hot label lhsT against an aux rhs whose columns carry
-(n_k - mean_n)/2 - BIG/2*onehot). The per-pair constant
  halfbias_p = (n_i - 2*u_p + A - mean_n)/2
is computed on HOST (O(pairs*D) numpy, same order as the gathers) so that
  relu(td) = 2*relu(V + halfbias_p)   and   td > 0  <=>  V > -halfbias_p.
ACT (scalar engine) does the relu row-sums, DVE (vector engine) the counts,
in parallel, one tile behind PE. Host sums the 8 cores' partials.
"""

import numpy as np
import ml_dtypes
from contextlib import ExitStack

import concourse.bass as bass
import concourse.mybir as mybir
from concourse.bass_utils import run_bass_kernel_spmd

F32 = mybir.dt.float32
BF16 = mybir.dt.bfloat16
AF = mybir.ActivationFunctionType
OP = mybir.AluOpType
NP_BF16 = ml_dtypes.bfloat16

N_CORES = 8
A_MARGIN = 0.2
BIG_HALF = 1.0e6  # BIG = 2e6 dominates any |td| (~1e3) by >>1e3x
PAD_NEG = -1.0e9  # halfbias for padding pair rows -> relu 0, count 0
MAX_TILES = 4     # per-core pair tiles per launch (PSUM bank budget)

_programs: dict = {}  # (T, N, L) -> bass.Bass
LAST_RES = None  # most recent BassKernelResults (for test harness tracing)


def _build_program(T: int, N: int, L: int) -> "bass.Bass":
    """One SPMD program: every core runs this with its own pair shard."""
    P = T * 128
    nc = bass.Bass()

    d_big = nc.declare_dram_parameter("big", [128, N + P], BF16, isOutput=False)
    d_aux = nc.declare_dram_parameter("aux", [L, P + N], BF16, isOutput=False)
    d_hcol = nc.declare_dram_parameter("hcol", [128, T], F32, isOutput=False)
    d_red = nc.declare_dram_parameter("red", [128, 2 * T], F32, isOutput=True)

    with ExitStack() as ctx:
        sb = lambda name, shape, dt: ctx.enter_context(nc.sbuf_tensor(name, shape, dt))
        ps = lambda name, shape: ctx.enter_context(nc.psum_tensor(name, shape, F32))

        # cols 0:N = emb^T (rhs of mm1), cols N:N+P = emb^T gathered by j (lhsT)
        big_sb = sb("big_sb", [128, N + P], BF16)
        # cols 0:P = one-hot(label_j) (lhsT of mm2), cols P:P+N = aux rhs
        aux_sb = sb("aux_sb", [L, P + N], BF16)
        hcol_sb = sb("hcol_sb", [128, T], F32)  # halfbias per tile column
        red_sb = sb("red_sb", [128, 2 * T], F32)    # relu sums | counts
        scr_a = [sb(f"scr_a{i}", [128, N], BF16) for i in range(2)]
        scr_d = sb("scr_d", [128, N], BF16)
        warm_sb = sb("warm_sb", [128, 1], BF16)
        psumV = [ps(f"psumV{t}", [128, N]) for t in range(T)]

        with (
            nc.semaphore("dma_in") as dma_in,
            nc.semaphore("hc") as hc,
            nc.semaphore("mm") as mm,
            nc.semaphore("act") as act,
            nc.semaphore("dve") as dve,
            nc.semaphore("dma_out") as dma_out,
            nc.Block() as block,
        ):

            @block.sync
            def _(sync):
                # big on the SP queue; aux+hcol go on the Act queue in
                # parallel (issued by the scalar engine) to halve DMA latency
                sync.dma_start(big_sb[:], d_big[:]).then_inc(dma_in, 16)
                sync.wait_ge(act, T)
                sync.wait_ge(dve, T)
                sync.dma_start(d_red[:], red_sb[:]).then_inc(dma_out, 16)
                sync.wait_ge(dma_out, 16)

            @block.tensor
            def _(tensor):
                tensor.wait_ge(dma_in, 32)
                for t in range(T):
                    nc.tensor.matmul(
                        psumV[t][:], big_sb[:, N + 128 * t:N + 128 * (t + 1)],
                        big_sb[:, 0:N], start=True, stop=False,
                    )
                    nc.tensor.matmul(
                        psumV[t][:], aux_sb[:, bass.ts(t, 128)],
                        aux_sb[:, P:P + N], start=False, stop=True,
                    ).then_inc(mm, 1)

            @block.scalar
            def _(scalar):
                scalar.dma_start(aux_sb[:], d_aux[:]).then_inc(dma_in, 16)
                scalar.dma_start(hcol_sb[:], d_hcol[:]).then_inc(hc, 16)
                # warm the Relu table while the big DMAs are in flight
                scalar.wait_ge(hc, 16)
                nc.scalar.activation(warm_sb[:], hcol_sb[:, 0:1], AF.Relu)
                for t in range(T):
                    scalar.wait_ge(mm, t + 1)
                    if t >= 2:
                        # DVE t-2 must be done reading scr_a[t%2]
                        scalar.wait_ge(dve, t - 1)
                    nc.scalar.activation(
                        scr_a[t % 2][:], psumV[t][:], AF.Relu,
                        bias=hcol_sb[:, t:t + 1],
                        accum_out=red_sb[:, t:t + 1],
                    ).then_inc(act, 1)

            @block.vector
            def _(vector):
                # count nonzeros of ACT's relu output (SBUF bf16: 2x DVE
                # rate, and no concurrent PSUM-bank access, which breaks HW)
                for t in range(T):
                    vector.wait_ge(act, t + 1)
                    if t:
                        vector.wait_ge(dve, t)
                    nc.vector.tensor_scalar(
                        scr_d[:], scr_a[t % 2][:], 0.0,
                        None, OP.is_gt, OP.add,
                        accum_out=red_sb[:, T + t:T + t + 1],
                    ).then_inc(dve, 1)

    return nc


def _get_program(T: int, N: int, L: int) -> "bass.Bass":
    key = (T, N, L)
    if key not in _programs:
        _programs[key] = _build_program(T, N, L)
    return _programs[key]


def _run_batch(shared, ii, jj, halfbias, T):
    """Run one SPMD launch over <=8*T*128 pairs; returns (sum, count) f64."""
    N, L, labels, emb_bf, embt_bf, auxr_bf = shared
    P = T * 128
    per = (len(ii) + N_CORES - 1) // N_CORES

    in_maps = []
    for c in range(N_CORES):
        sj = jj[c * per:(c + 1) * per]
        hb = halfbias[c * per:(c + 1) * per]
        m = len(sj)
        big = np.zeros((128, N + P), NP_BF16)
        big[:, :N] = embt_bf
        aux = np.zeros((L, P + N), NP_BF16)
        aux[:, P:] = auxr_bf
        hb_full = np.full(P, PAD_NEG, np.float32)
        if m:
            big[:, N:N + m] = emb_bf[sj].T
            aux[labels[sj], np.arange(m)] = 1.0
            hb_full[:m] = hb
        in_maps.append({
            "big": big,
            "aux": aux,
            "hcol": np.ascontiguousarray(hb_full.reshape(T, 128).T),
        })

    nc = _get_program(T, N, L)
    res = run_bass_kernel_spmd(nc, in_maps, list(range(N_CORES)))
    global LAST_RES
    LAST_RES = res
    s = 0.0
    cnt = 0.0
    for c in range(N_CORES):
        r = res.results[c]["red"].astype(np.float64)
        s += 2.0 * float(r[:, :T].sum())
        cnt += float(r[:, T:].sum())
    return s, cnt


def kernel(anchor, positive, negative, ind):
    anchor = np.asarray(anchor, dtype=np.float32)
    positive = np.asarray(positive, dtype=np.float32)
    negative = np.asarray(negative, dtype=np.float32)
    labels = np.asarray(ind).reshape(-1).astype(np.int64)

    emb = np.ascontiguousarray(np.concatenate([anchor, positive, negative], axis=0))
    N, D = emb.shape
    assert D == 128, f"kernel assumes D=128, got {D}"
    assert N == labels.shape[0]

    L = int(labels.max()) + 1 if labels.size else 1
    assert L <= 128, f"label ids must fit one-hot partitions, got {L}"

    # same-label (i, j) pairs, excluding the i=0 plane (keep[0] = False)
    same = labels[:, None] == labels[None, :]
    ii, jj = np.nonzero(same)
    sel = ii >= 1
    ii, jj = ii[sel].astype(np.int64), jj[sel].astype(np.int64)

    if len(ii) == 0:
        return np.float32(0.0)

    n = np.einsum("ij,ij->i", emb, emb, dtype=np.float64)
    mean_n = float(n.mean())
    u = np.einsum("ij,ij->i", emb[ii], emb[jj], dtype=np.float64)
    halfbias = ((n[ii] - 2.0 * u + A_MARGIN - mean_n) / 2.0).astype(np.float32)

    emb_bf = emb.astype(NP_BF16)
    embt_bf = np.ascontiguousarray(emb_bf.T)
    auxr = np.tile((-(n - mean_n) / 2.0).astype(np.float32), (L, 1))
    auxr[labels, np.arange(N)] -= BIG_HALF
    auxr_bf = auxr.astype(NP_BF16)
    shared = (N, L, labels, emb_bf, embt_bf, auxr_bf)

    batch_cap = N_CORES * MAX_TILES * 128
    s_tot, c_tot = 0.0, 0.0
    for b0 in range(0, len(ii), batch_cap):
        bi = ii[b0:b0 + batch_cap]
        bj = jj[b0:b0 + batch_cap]
        hb = halfbias[b0:b0 + batch_cap]
        per = (len(bi) + N_CORES - 1) // N_CORES
        T = max(1, (per + 127) // 128)
        s, c = _run_batch(shared, bi, bj, hb, T)
        s_tot += s
        c_tot += c

    if c_tot > 0:
        return np.float32(s_tot / max(c_tot, 1.0))
    return np.float32(0.0)


# revision 35
# speedup vs baseline: 2.0572x; 1.0584x over previous
"""HardTripletMiningLoss on 8 TRN2 NeuronCores (Bass, raw-block SPMD).

Math: with emb = concat(anchor, positive, negative) [N,D], labels = ind[:,0],
pd(a,b) = ||e_a - e_b||^2, the loss is the mean over triplets (i,j,k) of
td = pd(i,j) - pd(j,k) + A restricted to
  same(i,j) & ~same(j,k) & td > 0 & i != 0.
Only (i,j) pairs with same labels (and i>=1) contribute — ~N^2/L of N^2 pairs.
Each such pair p=(i,j) needs, over k: sum/count of relu(td), where
  td(p,k) = 2*g(j,k) - n_k + (n_i - 2*u_p + A),  u_p = <e_i, e_j>.

Device work per core (pairs sharded 8 ways, rows of [128, N] tiles):
  V[p,k] = g(j_p,k) - (n_k - mean_n)/2 - (BIG/2)*same(j_p,k)
via two accumulating bf16 matmuls per tile (emb^T gathered by j against
emb^T, then a one-hot label lhsT against an aux rhs whose columns carry
-(n_k - mean_n)/2 - BIG/2*onehot). The per-pair constant
  halfbias_p = (n_i - 2*u_p + A - mean_n)/2
is computed on HOST (O(pairs*D) numpy, same order as the gathers) so that
  relu(td) = 2*relu(V + halfbias_p)   and   td > 0  <=>  V > -halfbias_p.
ACT (scalar engine) does the relu row-sums, DVE (vector engine) the counts,
in parallel, one tile behind PE. Host sums the 8 cores' partials.
"""

import numpy as np
import ml_dtypes
from contextlib import ExitStack

import concourse.bass as bass
import concourse.mybir as mybir
from concourse.bass_utils import run_bass_kernel_spmd

F32 = mybir.dt.float32
BF16 = mybir.dt.bfloat16
AF = mybir.ActivationFunctionType
OP = mybir.AluOpType
NP_BF16 = ml_dtypes.bfloat16

N_CORES = 8
A_MARGIN = 0.2
BIG_HALF = 1.0e6  # BIG = 2e6 dominates any |td| (~1e3) by >>1e3x
PAD_NEG = -1.0e9  # halfbias for padding pair rows -> relu 0, count 0
MAX_TILES = 4     # per-core pair tiles per launch (PSUM bank budget)

_programs: dict = {}  # (T, N, L) -> bass.Bass
LAST_RES = None  # most recent BassKernelResults (for test harness tracing)


def _build_program(T: int, N: int, L: int) -> "bass.Bass":
    """One SPMD program: every core runs this with its own pair shard."""
    P = T * 128
    nc = bass.Bass()

    d_big = nc.declare_dram_parameter("big", [128, N + P], BF16, isOutput=False)
    d_aux = nc.declare_dram_parameter("aux", [L, P + N], BF16, isOutput=False)
    d_hcol = nc.declare_dram_parameter("hcol", [128, T], F32, isOutput=False)
    d_red = nc.declare_dram_parameter("red", [128, 2 * T], F32, isOutput=True)

    with ExitStack() as ctx:
        sb = lambda name, shape, dt: ctx.enter_context(nc.sbuf_tensor(name, shape, dt))
        ps = lambda name, shape: ctx.enter_context(nc.psum_tensor(name, shape, F32))

        # cols 0:N = emb^T (rhs of mm1), cols N:N+P = emb^T gathered by j (lhsT)
        big_sb = sb("big_sb", [128, N + P], BF16)
        # cols 0:P = one-hot(label_j) (lhsT of mm2), cols P:P+N = aux rhs
        aux_sb = sb("aux_sb", [L, P + N], BF16)
        hcol_sb = sb("hcol_sb", [128, T], F32)  # halfbias per tile column
        red_sb = sb("red_sb", [128, 2 * T], F32)    # relu sums | counts
        scr_a = [sb(f"scr_a{i}", [128, N], BF16) for i in range(2)]
        scr_d = sb("scr_d", [128, N], BF16)
        warm_sb = sb("warm_sb", [128, 1], BF16)
        psumV = [ps(f"psumV{t}", [128, N]) for t in range(T)]

        with (
            nc.semaphore("dma_in") as dma_in,
            nc.semaphore("auxs") as auxs,
            nc.semaphore("hc") as hc,
            nc.semaphore("mm") as mm,
            nc.semaphore("act") as act,
            nc.semaphore("cnt") as cnt,
            nc.semaphore("dma_out") as dma_out,
            nc.Block() as block,
        ):

            @block.sync
            def _(sync):
                # Three parallel DMA lanes: SP carries embt, the Act queue
                # carries embjt + aux, Pool (SWDGE) carries hcol.
                sync.dma_start(big_sb[:, 0:N], d_big[:, 0:N]).then_inc(dma_in, 16)
                sync.wait_ge(cnt, T)
                sync.dma_start(d_red[:], red_sb[:]).then_inc(dma_out, 16)
                sync.wait_ge(dma_out, 16)

            @block.tensor
            def _(tensor):
                tensor.wait_ge(dma_in, 32)
                for t in range(T):
                    nc.tensor.matmul(
                        psumV[t][:], big_sb[:, N + 128 * t:N + 128 * (t + 1)],
                        big_sb[:, 0:N], start=True, stop=False,
                    )
                    if t == 0:
                        tensor.wait_ge(auxs, 16)
                    nc.tensor.matmul(
                        psumV[t][:], aux_sb[:, bass.ts(t, 128)],
                        aux_sb[:, P:P + N], start=False, stop=True,
                    ).then_inc(mm, 1)

            @block.scalar
            def _(scalar):
                scalar.dma_start(
                    big_sb[:, N:N + P], d_big[:, N:N + P]).then_inc(dma_in, 16)
                scalar.dma_start(aux_sb[:], d_aux[:]).then_inc(auxs, 16)
                # warm the Relu table while the big DMAs are in flight
                scalar.wait_ge(hc, 16)
                nc.scalar.activation(warm_sb[:], hcol_sb[:, 0:1], AF.Relu)
                for t in range(T):
                    scalar.wait_ge(mm, t + 1)
                    if t >= 2:
                        # counter (Pool) must be done reading scr_a[t%2]
                        scalar.wait_ge(cnt, t - 1)
                    nc.scalar.activation(
                        scr_a[t % 2][:], psumV[t][:], AF.Relu,
                        bias=hcol_sb[:, t:t + 1],
                        accum_out=red_sb[:, t:t + 1],
                    ).then_inc(act, 1)

            @block.gpsimd
            def _(gpsimd):
                gpsimd.dma_start(hcol_sb[:], d_hcol[:]).then_inc(hc, 16)

            @block.vector
            def _(vector):
                # count nonzeros of ACT's relu output (SBUF, not PSUM —
                # concurrent PSUM-bank access from two engines breaks HW)
                for t in range(T):
                    vector.wait_ge(act, t + 1)
                    if t:
                        vector.wait_ge(cnt, t)
                    nc.vector.tensor_scalar(
                        scr_d[:], scr_a[t % 2][:], 0.0,
                        None, OP.is_gt, OP.add,
                        accum_out=red_sb[:, T + t:T + t + 1],
                    ).then_inc(cnt, 1)

    return nc


def _get_program(T: int, N: int, L: int) -> "bass.Bass":
    key = (T, N, L)
    if key not in _programs:
        _programs[key] = _build_program(T, N, L)
    return _programs[key]


def _run_batch(shared, ii, jj, halfbias, T):
    """Run one SPMD launch over <=8*T*128 pairs; returns (sum, count) f64."""
    N, L, labels, emb_bf, embt_bf, auxr_bf = shared
    P = T * 128
    per = (len(ii) + N_CORES - 1) // N_CORES

    in_maps = []
    for c in range(N_CORES):
        sj = jj[c * per:(c + 1) * per]
        hb = halfbias[c * per:(c + 1) * per]
        m = len(sj)
        big = np.zeros((128, N + P), NP_BF16)
        big[:, :N] = embt_bf
        aux = np.zeros((L, P + N), NP_BF16)
        aux[:, P:] = auxr_bf
        hb_full = np.full(P, PAD_NEG, np.float32)
        if m:
            big[:, N:N + m] = emb_bf[sj].T
            aux[labels[sj], np.arange(m)] = 1.0
            hb_full[:m] = hb
        in_maps.append({
            "big": big,
            "aux": aux,
            "hcol": np.ascontiguousarray(hb_full.reshape(T, 128).T),
        })

    nc = _get_program(T, N, L)
    res = run_bass_kernel_spmd(nc, in_maps, list(range(N_CORES)))
    global LAST_RES
    LAST_RES = res
    s = 0.0
    cnt = 0.0
    for c in range(N_CORES):
        r = res.results[c]["red"].astype(np.float64)
        s += 2.0 * float(r[:, :T].sum())
        cnt += float(r[:, T:].sum())
    return s, cnt


def kernel(anchor, positive, negative, ind):
    anchor = np.asarray(anchor, dtype=np.float32)
    positive = np.asarray(positive, dtype=np.float32)
    negative = np.asarray(negative, dtype=np.float32)
    labels = np.asarray(ind).reshape(-1).astype(np.int64)

    emb = np.ascontiguousarray(np.concatenate([anchor, positive, negative], axis=0))
    N, D = emb.shape
    assert D == 128, f"kernel assumes D=128, got {D}"
    assert N == labels.shape[0]

    L = int(labels.max()) + 1 if labels.size else 1
    assert L <= 128, f"label ids must fit one-hot partitions, got {L}"

    # same-label (i, j) pairs, excluding the i=0 plane (keep[0] = False)
    same = labels[:, None] == labels[None, :]
    ii, jj = np.nonzero(same)
    sel = ii >= 1
    ii, jj = ii[sel].astype(np.int64), jj[sel].astype(np.int64)

    if len(ii) == 0:
        return np.float32(0.0)

    n = np.einsum("ij,ij->i", emb, emb, dtype=np.float64)
    mean_n = float(n.mean())
    u = np.einsum("ij,ij->i", emb[ii], emb[jj], dtype=np.float64)
    halfbias = ((n[ii] - 2.0 * u + A_MARGIN - mean_n) / 2.0).astype(np.float32)

    emb_bf = emb.astype(NP_BF16)
    embt_bf = np.ascontiguousarray(emb_bf.T)
    auxr = np.tile((-(n - mean_n) / 2.0).astype(np.float32), (L, 1))
    auxr[labels, np.arange(N)] -= BIG_HALF
    auxr_bf = auxr.astype(NP_BF16)
    shared = (N, L, labels, emb_bf, embt_bf, auxr_bf)

    batch_cap = N_CORES * MAX_TILES * 128
    s_tot, c_tot = 0.0, 0.0
    for b0 in range(0, len(ii), batch_cap):
        bi = ii[b0:b0 + batch_cap]
        bj = jj[b0:b0 + batch_cap]
        hb = halfbias[b0:b0 + batch_cap]
        per = (len(bi) + N_CORES - 1) // N_CORES
        T = max(1, (per + 127) // 128)
        s, c = _run_batch(shared, bi, bj, hb, T)
        s_tot += s
        c_tot += c

    if c_tot > 0:
        return np.float32(s_tot / max(c_tot, 1.0))
    return np.float32(0.0)
